# revision 1
# baseline (speedup 1.0000x reference)
"""ChebConv (order-4) GNN layer on 8 Trainium2 NeuronCores.

Reference computation (fp32):
    T0 = x, T1 = G x, Tk = 2 G T{k-1} - T{k-2}
    out = sum_k Tk @ W[k]          # [N, F] with N=10000, F=32

Strategy:
  * Rewrite in the power basis: y0 = x, yk = G y{k-1},
      out = sum_k yk @ Wp[k]  with
      Wp = [W0 - W2, W1 - 3 W3, 2 W2, 4 W3]   (exact modulo fp reassociation)
    so each hop is a bare matmul against G (no 2*/- epilogue).
  * Row-shard G over 8 cores (1280 padded rows each). The per-core lhsT
    tiles must hold G^T, so the host passes each core a contiguous
    transposed slice (pad N 10000 -> 10240).
  * fp32 matmuls on the TRN2 PE run in LOW_HIGH mode: 2 passes, each
    streaming the fp32 rhs at half rate (4x bf16 cost). Instead we do a
    software hi/lo split: G = G_hi + G_lo and v = v_hi + v_lo (bf16
    pairs) and compute G_hi v_hi + G_lo v_hi + G_hi v_lo with fp32 PSUM
    accumulation -- 3 full-rate bf16 passes, same DRAM bytes as fp32,
    ~7e-6 relative error (vs 3e-3 for plain bf16). Per fc sweep, G_hi
    and G_lo rows are interleaved in one [NP, 2*l] array so each
    128-row j-chunk is a single contiguous DMA.
  * Each hop runs as 3 sweeps, one per <=512-column chunk of yk^T.
    Per sweep and 128-row j-chunk: matmuls (lhsT=v_{hi,lo}[j-chunk]
    [128,32] bf16, rhs=G^T_{hi,lo} tile [128,<=512] bf16) accumulate
    the sweep's [32,<=512] chunk of yk^T over all 80 j-chunks (one open
    PSUM accumulation group per bank). The last (smallest) sweep's G
    block stays pinned in SBUF across hops (10.5 MB saved twice).
  * The Wp contraction happens on-chip from yk^T in full fp32:
    matmul(lhsT=Wp_k [32,32], rhs=ykT chunk), DVE-add into the
    transposed output accumulator; the k=0 term uses the host xT slice.
  * After each sweep (except in the last hop), its rows are
    PE-transposed ([32,128] -> [128,32] blocks) into natural m-chunk
    layout, split into bf16 hi/lo, and all-gathered in a partial
    collective (DRAM bounce) that overlaps the remaining sweeps. The
    reload into the next hop's per-part v tiles rides SWDGE (gpsimd) so
    the gather-gated DMA cannot convoy the G stream on the shared HWDGE
    completion lanes; j-chunks are consumed in gather-firing order so
    each hop starts on columns whose gather finished first.
  * Output is returned transposed ([32, 1280] per core); the host
    concatenates, transposes and drops padding.
"""

import sys

if "/opt/trn_rl_repo" not in sys.path:
    sys.path.insert(0, "/opt/trn_rl_repo")

import numpy as np

N = 10000
F = 32
ORDER = 4
NCORES = 8
P = 128
NP = 10240  # padded node count: divisible by NCORES * P
RPC = NP // NCORES  # rows per core (1280)
JC = NP // P  # global 128-row chunks (80)
MC = RPC // P  # local 128-row chunks per core (10)

_CACHE = {}


def _build(np_total, ncores):
    from concourse import bacc, masks, mybir, tile

    rpc = np_total // ncores
    jc = np_total // P
    mc = rpc // P
    f32 = mybir.dt.float32
    bf16 = mybir.dt.bfloat16
    fchunks = [(s, min(512, rpc - s)) for s in range(0, rpc, 512)]
    nfc = len(fchunks)

    nc = bacc.Bacc(
        "TRN2", target_bir_lowering=False, debug=False, num_devices=ncores
    )
    # one G^T block per fc sweep, rows = [hi cols | lo cols] interleaved
    ghls = [
        nc.dram_tensor(f"ghl{i}", [np_total, 2 * l], bf16, kind="ExternalInput").ap()
        for i, (s, l) in enumerate(fchunks)
    ]
    # per-part m-chunk geometry: part i covers m-chunks [m0, m0+nm)
    parts = [(s // P, l // P) for s, l in fchunks]
    # x in per-part v layout: concat over parts of [hi block | lo block],
    # block col (c*nm + ml)*F + f = padded x row (c*mc + m0 + ml)*P + p
    vcols = [2 * ncores * nm * F for (m0, nm) in parts]
    xthl = nc.dram_tensor("xthl", [P, sum(vcols)], bf16, kind="ExternalInput").ap()
    xt = nc.dram_tensor("xt", [F, rpc], f32, kind="ExternalInput").ap()
    wp = nc.dram_tensor("wp", [F, ORDER * F], f32, kind="ExternalInput").ap()
    out_t = nc.dram_tensor("outT", [F, rpc], f32, kind="ExternalOutput").ap()

    # pin the last (smallest) fc sweep's G block in SBUF across hops
    pin_i = nfc - 1
    pin_l = fchunks[pin_i][1]

    def part_of(m):
        for i, (m0, nm) in enumerate(parts):
            if m0 <= m < m0 + nm:
                return i
        raise AssertionError

    with tile.TileContext(nc) as tc:
        with (
            tc.tile_pool(name="const", bufs=1) as constp,
            tc.tile_pool(name="gtp", bufs=10) as gtp,
            tc.tile_pool(name="vp", bufs=2) as vp,
            tc.tile_pool(name="sb", bufs=2) as sb,
            tc.tile_pool(name="ps_hop", bufs=1, space="PSUM") as ps_hop,
            tc.tile_pool(name="ps_tp", bufs=2, space="PSUM") as ps_tp,
            tc.tile_pool(name="ps_w", bufs=2, space="PSUM") as ps_w,
            tc.tile_pool(name="dram", bufs=2, space="DRAM") as dram,
        ):
            ident = constp.tile([P, P], f32)
            masks.make_identity(nc, ident[:])
            w_sb = constp.tile([F, ORDER * F], f32)
            nc.scalar.dma_start(w_sb[:], wp)
            xt_sb = constp.tile([F, rpc], f32)
            nc.scalar.dma_start(xt_sb[:], xt)
            out_sb = constp.tile([F, rpc], f32)
            pin = constp.tile([P, jc * 2 * pin_l], bf16)

            # v holds y_{k-1} as bf16 hi/lo pairs, one tile per fc part so
            # next-hop matmuls only depend on the partial gather that
            # produced their columns
            v_parts = []
            off = 0
            for i, w_ in enumerate(vcols):
                vt = vp.tile([P, w_], bf16, tag=f"v{i}", name=f"v{i}")
                nc.scalar.dma_start(vt[:], xthl[:, off : off + w_])
                off += w_
                v_parts.append(vt)

            def v_hi(vps, j):
                c, m = j // mc, j % mc
                i = part_of(m)
                m0, nm = parts[i]
                col = (c * 2 * nm + (m - m0)) * F
                return vps[i][:, col : col + F]

            def v_lo(vps, j):
                c, m = j // mc, j % mc
                i = part_of(m)
                m0, nm = parts[i]
                col = (c * 2 * nm + nm + (m - m0)) * F
                return vps[i][:, col : col + F]

            # k = 0 contribution: out^T = Wp_0^T @ x^T (pure fp32)
            for s, l in fchunks:
                pw = ps_w.tile([F, l], f32, tag="pw")
                nc.tensor.matmul(
                    pw[:], lhsT=w_sb[:, 0:F], rhs=xt_sb[:, s : s + l],
                    start=True, stop=True,
                )
                nc.vector.tensor_copy(out_sb[:, s : s + l], pw[:])

            # j-chunks are consumed in sweep (= gather-firing) order so
            # each hop starts on columns whose gather finished first; the
            # pinned sweep stays last: its gather is smallest and its
            # consumers come after a ~46us runway in the next hop
            sweep_order = list(range(nfc))
            part_rank = {i: r for r, i in enumerate(sweep_order)}
            jorder = sorted(range(jc), key=lambda j: (part_rank[part_of(j % mc)], j))

            def reload_v(i, cc_out, v_dst):
                # SWDGE (gpsimd) so the gather-gated reload can't convoy
                # the G stream on the shared HWDGE completion lanes; one
                # DMA per part (hi/lo interleaved per core block)
                nc.gpsimd.dma_start(
                    v_dst[i][:].rearrange("p (c m) -> p c m", c=ncores),
                    cc_out[:].rearrange("(c p) m -> p c m", p=P),
                )

            for k in range(1, ORDER):
                v_cur = v_parts
                if k < ORDER - 1:
                    v_next = [
                        vp.tile([P, w_], bf16, tag=f"v{i}", name=f"vn{i}")
                        for i, w_ in enumerate(vcols)
                    ]
                y_t = sb.tile([F, rpc], f32, tag="yT")
                js = jorder
                # hop: y_k^T = (G @ y_{k-1})^T via 3 bf16 hi/lo passes,
                # one sweep per fc chunk so partial all-gathers overlap
                # the remaining sweeps
                for i in sweep_order:
                    s, l = fchunks[i]
                    # when both hi and lo rhs fit one PSUM bank, fuse the
                    # two v_hi passes into a single 2l-column matmul and
                    # fold the halves with the epilogue DVE op instead
                    merged = 2 * l <= 512
                    hp = ps_hop.tile(
                        [F, 2 * l] if merged else [F, l],
                        f32, tag=f"hop{i}", name=f"hp{i}",
                    )
                    pinned = i == pin_i
                    for jn, j in enumerate(js):
                        if pinned:
                            g = pin[:, j * 2 * l : (j + 1) * 2 * l]
                            if k == 1:
                                nc.sync.dma_start(
                                    g, ghls[i][j * P : (j + 1) * P, :]
                                )
                        else:
                            gt = gtp.tile(
                                [P, 2 * l], bf16, tag=f"gt{i}", name="gt"
                            )
                            nc.sync.dma_start(
                                gt[:], ghls[i][j * P : (j + 1) * P, :]
                            )
                            g = gt[:]
                        gh = g[:, 0:l]
                        gl = g[:, l : 2 * l]
                        if merged:
                            nc.tensor.matmul(
                                hp[:], lhsT=v_hi(v_cur, j), rhs=g[:, 0 : 2 * l],
                                start=(jn == 0), stop=False,
                            )
                            nc.tensor.matmul(
                                hp[:, 0:l], lhsT=v_lo(v_cur, j), rhs=gh,
                                start=False, stop=(jn == jc - 1),
                            )
                        else:
                            for t, (lhs, rhs) in enumerate(
                                (
                                    (v_hi(v_cur, j), gh),
                                    (v_lo(v_cur, j), gh),
                                    (v_hi(v_cur, j), gl),
                                )
                            ):
                                nc.tensor.matmul(
                                    hp[:], lhsT=lhs, rhs=rhs,
                                    start=(jn == 0 and t == 0),
                                    stop=(jn == jc - 1 and t == 2),
                                )
                    # sweep epilogue: copy out (folding the merged
                    # halves), Wp contribution
                    if merged:
                        # walrus allows only one PSUM operand per DVE op
                        nc.vector.tensor_copy(y_t[:, s : s + l], hp[:, 0:l])
                        nc.vector.tensor_add(
                            y_t[:, s : s + l], y_t[:, s : s + l], hp[:, l : 2 * l]
                        )
                    else:
                        nc.vector.tensor_copy(y_t[:, s : s + l], hp[:])
                    pw = ps_w.tile([F, l], f32, tag="pw")
                    nc.tensor.matmul(
                        pw[:], lhsT=w_sb[:, k * F : (k + 1) * F],
                        rhs=y_t[:, s : s + l], start=True, stop=True,
                    )
                    nc.vector.tensor_add(
                        out_sb[:, s : s + l], out_sb[:, s : s + l], pw[:]
                    )
                    if k < ORDER - 1:
                        # transpose this sweep's rows to natural layout,
                        # split bf16 hi/lo, partial all-gather; the
                        # reload into the next hop's v happens there
                        m0, nm = parts[i]
                        stage = sb.tile(
                            [P, 2 * nm * F], bf16, tag=f"stage{i}",
                            name=f"stage{i}",
                        )
                        for mm in range(nm):
                            m = m0 + mm
                            tp = ps_tp.tile([P, F], f32, tag="tp", name="tp")
                            nc.tensor.transpose(
                                tp[:], y_t[:, m * P : (m + 1) * P],
                                ident[0:F, 0:F],
                            )
                            hi = stage[:, mm * F : (mm + 1) * F]
                            lo = stage[:, (nm + mm) * F : (nm + mm + 1) * F]
                            nc.vector.tensor_copy(hi, tp[:])
                            nc.vector.tensor_sub(lo, tp[:], hi)
                        cc_in = dram.tile(
                            [P, 2 * nm * F], bf16, tag=f"ccin{i}",
                            name=f"ccin{i}",
                        )
                        cc_out = dram.tile(
                            [ncores * P, 2 * nm * F], bf16, tag=f"ccout{i}",
                            name=f"ccout{i}",
                        )
                        nc.scalar.dma_start(cc_in[:], stage[:])
                        nc.gpsimd.collective_compute(
                            "AllGather",
                            mybir.AluOpType.bypass,
                            replica_groups=[list(range(ncores))],
                            ins=[cc_in.opt()],
                            outs=[cc_out.opt()],
                        )
                        reload_v(i, cc_out, v_next)
                if k < ORDER - 1:
                    v_parts = v_next

            nc.scalar.dma_start(out_t, out_sb[:])

    nc.compile()
    return nc


def get_nc(np_total=NP, ncores=NCORES):
    key = (np_total, ncores)
    if key not in _CACHE:
        _CACHE[key] = _build(np_total, ncores)
    return _CACHE[key]


def _bf16_pair(a):
    import ml_dtypes

    hi = a.astype(ml_dtypes.bfloat16)
    lo = (a - hi.astype(np.float32)).astype(ml_dtypes.bfloat16)
    return hi, lo


def prep_inputs(x, gso, weight, np_total=NP, ncores=NCORES):
    """Host-side shard prep. Returns in_maps for run_bass_kernel_spmd."""
    n = x.shape[0]
    rpc = np_total // ncores
    jc = np_total // P

    x = np.asarray(x, dtype=np.float32)
    gso = np.asarray(gso, dtype=np.float32)
    weight = np.asarray(weight, dtype=np.float32)

    wp = np.concatenate(
        [
            weight[0] - weight[2],
            weight[1] - 3.0 * weight[3],
            2.0 * weight[2],
            4.0 * weight[3],
        ],
        axis=1,
    ).astype(np.float32)  # [F, ORDER*F]

    xpad = np.zeros((np_total, F), dtype=np.float32)
    xpad[:n] = x
    gpad = np.zeros((np_total, np_total), dtype=np.float32)
    gpad[:n, :n] = gso
    g_hi, g_lo = _bf16_pair(gpad)

    # x as bf16 hi/lo pair in the per-part v layout:
    # for part (m0, nm): block col (c*nm + ml)*F + f = row (c*mc+m0+ml)*P + p
    x_hi, x_lo = _bf16_pair(xpad)
    mc = rpc // P
    parts = [(s // P, min(512, rpc - s) // P) for s in range(0, rpc, 512)]

    def part_x(m0, nm):
        # [P, (c, hi|lo, ml, f)] interleaved per core block
        hi = x_hi.reshape(ncores, mc, P, F)[:, m0 : m0 + nm].transpose(2, 0, 1, 3)
        lo = x_lo.reshape(ncores, mc, P, F)[:, m0 : m0 + nm].transpose(2, 0, 1, 3)
        return np.stack([hi, lo], axis=2).reshape(P, ncores * 2 * nm * F)

    xthl = np.ascontiguousarray(
        np.concatenate([part_x(m0, nm) for (m0, nm) in parts], axis=1)
    )

    fchunks = [(s, min(512, rpc - s)) for s in range(0, rpc, 512)]
    in_maps = []
    for c in range(ncores):
        rows = slice(c * rpc, (c + 1) * rpc)
        ght_c = g_hi[rows, :].T  # [np_total, rpc] bf16
        glt_c = g_lo[rows, :].T
        m = {"xthl": xthl, "wp": wp}
        m["xt"] = np.ascontiguousarray(xpad[rows, :].T)  # [F, rpc] fp32
        for i, (s, l) in enumerate(fchunks):
            # per-row [hi cols | lo cols] for this fc sweep
            m[f"ghl{i}"] = np.ascontiguousarray(
                np.concatenate(
                    [ght_c[:, s : s + l], glt_c[:, s : s + l]], axis=1
                )
            )
        in_maps.append(m)
    return in_maps


def assemble_output(results, n=N, ncores=NCORES):
    out_t = np.concatenate([results[c]["outT"] for c in range(ncores)], axis=1)
    return np.ascontiguousarray(out_t.T[:n]).astype(np.float32)


def kernel(x, gso, weight):
    import time

    from concourse import bass_utils

    nc = get_nc()
    in_maps = prep_inputs(x, gso, weight)
    last_err = None
    for attempt in range(3):
        try:
            res = bass_utils.run_bass_kernel_spmd(
                nc, in_maps, core_ids=list(range(NCORES))
            )
            return assemble_output(res.results)
        except Exception as e:  # transient device wedge: retry
            last_err = e
            time.sleep(5.0 * (attempt + 1))
    raise last_err



# revision 3
# speedup vs baseline: 2.0526x; 2.0526x over previous
"""ChebConv (order-4) GNN layer on 8 Trainium2 NeuronCores.

Reference computation (fp32):
    T0 = x, T1 = G x, Tk = 2 G T{k-1} - T{k-2}
    out = sum_k Tk @ W[k]          # [N, F] with N=10000, F=32

Strategy:
  * Rewrite in the power basis: y0 = x, yk = G y{k-1},
      out = sum_k yk @ Wp[k]  with
      Wp = [W0 - W2, W1 - 3 W3, 2 W2, 4 W3]   (exact modulo fp reassociation)
    so each hop is a bare matmul against G (no 2*/- epilogue).
  * Row-shard G over 8 cores (1280 padded rows each). The per-core lhsT
    tiles must hold G^T, so the host passes each core contiguous
    transposed slices (pad N 10000 -> 10240).
  * Everything runs in plain bf16 (fp32 PSUM accumulation): measured
    end-to-end relative error ~3.3e-3 against the fp64 oracle, well
    inside the 2e-2 gate.  One bf16 pass per hop is 3x less PE
    streaming and 2x less HBM than the previous fp32-accurate hi/lo
    scheme.
  * Each hop runs as 3 sweeps over column chunks (512, 384, 384) of
    yk^T.  Chunks 0 and 2 stay pinned in SBUF across hops (~140KB per
    partition); chunk 1 streams from DRAM each hop, hidden under the
    PE time of the surrounding sweeps.  Per sweep: matmuls
    (lhsT=v[j-chunk] [128,32] bf16, rhs=G^T tile [128,l] bf16)
    accumulate the sweep's [32,l] chunk of yk^T over all 79 valid
    128-row j-chunks in one open PSUM accumulation group.  The
    all-padding j-chunk 79 is skipped.
  * G tiles ride supertile DMAs: runs of up to 4 consecutive j-chunks
    (contiguous 512 DRAM rows) land in one descriptor, rearranged
    (t p) c -> p (t c) into SBUF.
  * The k=0 term (x @ Wp0)^T is precomputed on the host and DMAed
    straight into the output accumulator.  The k>=1 Wp contraction
    happens on-chip from yk^T in fp32: matmul(lhsT=Wp_k [32,32],
    rhs=ykT chunk), DVE-add into the transposed output accumulator.
  * After each sweep (except in the last hop), its rows are
    PE-transposed ([32,128] -> [128,32] blocks) into natural m-chunk
    layout, cast bf16, and all-gathered in a partial collective (DRAM
    bounce) that overlaps the remaining sweeps.  The reload into the
    next hop's per-part v tiles rides SWDGE (gpsimd) so the
    gather-gated DMA cannot convoy the G stream on the shared HWDGE
    completion lanes; j-chunks are consumed in gather-firing order so
    each hop starts on columns whose gather finished first.
  * Output is returned transposed ([32, 1280] per core); the host
    concatenates, transposes and drops padding.
"""

import sys

if "/opt/trn_rl_repo" not in sys.path:
    sys.path.insert(0, "/opt/trn_rl_repo")

import numpy as np

N = 10000
F = 32
ORDER = 4
NCORES = 8
P = 128
NP = 10240  # padded node count: divisible by NCORES * P
RPC = NP // NCORES  # rows per core (1280)
JC = NP // P  # global 128-row chunks (80); the last is all padding
MC = RPC // P  # local 128-row chunks per core (10)

# column chunks of the per-core output slice: (start, len)
FCHUNKS = [(0, 512), (512, 384), (896, 384)]
# per-part m-chunk geometry: part i covers m-chunks [m0, m0+nm)
PARTS = [(0, 4), (4, 3), (7, 3)]
STREAM_I = 1  # chunk 1 streams from DRAM each hop; 0 and 2 are pinned

_CACHE = {}


def _runs(jc, mc):
    """j-chunk runs in consumption (= gather-firing) order, skipping the
    all-padding final chunk jc-1."""
    runs = []
    for (m0, nm) in PARTS:
        for c in range(NCORES):
            j0 = c * mc + m0
            ln = nm
            if j0 + ln > jc - 1:
                ln = (jc - 1) - j0
            if ln > 0:
                runs.append((j0, ln))
    return runs


def _build(np_total, ncores):
    from concourse import bacc, masks, mybir, tile

    rpc = np_total // ncores
    jc = np_total // P
    mc = rpc // P
    f32 = mybir.dt.float32
    bf16 = mybir.dt.bfloat16
    jcv = jc - 1  # valid j-chunks

    nc = bacc.Bacc(
        "TRN2", target_bir_lowering=False, debug=False, num_devices=ncores
    )
    # one G^T block per fc sweep (bf16 columns of the core's row slice)
    gs = [
        nc.dram_tensor(f"g{i}", [np_total, l], bf16, kind="ExternalInput").ap()
        for i, (s, l) in enumerate(FCHUNKS)
    ]
    vcols = [ncores * nm * F for (m0, nm) in PARTS]
    # x in per-part v layout: block col (c*nm + ml)*F + f
    #   = padded x row (c*mc + m0 + ml)*P + p
    xtb = nc.dram_tensor("xtb", [P, sum(vcols)], bf16, kind="ExternalInput").ap()
    # host-precomputed k=0 term (x @ Wp0)^T for this core's rows
    out0 = nc.dram_tensor("out0", [F, rpc], f32, kind="ExternalInput").ap()
    wp = nc.dram_tensor("wp", [F, (ORDER - 1) * F], f32, kind="ExternalInput").ap()
    out_t = nc.dram_tensor("outT", [F, rpc], f32, kind="ExternalOutput").ap()

    runs = _runs(jc, mc)
    njv = sum(ln for _, ln in runs)
    assert njv == jcv

    def part_of(m):
        for i, (m0, nm) in enumerate(PARTS):
            if m0 <= m < m0 + nm:
                return i
        raise AssertionError

    with tile.TileContext(nc) as tc:
        with (
            tc.tile_pool(name="const", bufs=1) as constp,
            tc.tile_pool(name="gtp", bufs=5) as gtp,
            tc.tile_pool(name="vp", bufs=2) as vp,
            tc.tile_pool(name="sb", bufs=2) as sb,
            tc.tile_pool(name="ps_hop", bufs=1, space="PSUM") as ps_hop,
            tc.tile_pool(name="ps_tp", bufs=2, space="PSUM") as ps_tp,
            tc.tile_pool(name="ps_w", bufs=2, space="PSUM") as ps_w,
            tc.tile_pool(name="dram", bufs=2, space="DRAM") as dram,
        ):
            ident = constp.tile([P, P], f32)
            masks.make_identity(nc, ident[:])
            w_sb = constp.tile([F, (ORDER - 1) * F], f32)
            nc.scalar.dma_start(w_sb[:], wp)
            out_sb = constp.tile([F, rpc], f32)
            nc.scalar.dma_start(out_sb[:], out0)
            pins = {
                i: constp.tile(
                    [P, jcv * FCHUNKS[i][1]], bf16, name=f"pin{i}"
                )
                for i in range(len(FCHUNKS))
                if i != STREAM_I
            }

            # v holds y_{k-1} in bf16, one tile per fc part so next-hop
            # matmuls only depend on the partial gather that produced
            # their columns
            v_parts = []
            off = 0
            for i, w_ in enumerate(vcols):
                vt = vp.tile([P, w_], bf16, tag=f"v{i}", name=f"v{i}")
                nc.scalar.dma_start(vt[:], xtb[:, off : off + w_])
                off += w_
                v_parts.append(vt)

            def v_of(vps, j):
                c, m = j // mc, j % mc
                i = part_of(m)
                m0, nm = PARTS[i]
                col = (c * nm + (m - m0)) * F
                return vps[i][:, col : col + F]

            for k in range(1, ORDER):
                v_cur = v_parts
                if k < ORDER - 1:
                    v_next = [
                        vp.tile([P, w_], bf16, tag=f"v{i}", name=f"vn{i}_{k}")
                        for i, w_ in enumerate(vcols)
                    ]
                y_t = sb.tile([F, rpc], f32, tag="yT")
                # hop: y_k^T = (G @ y_{k-1})^T, one sweep per fc chunk so
                # partial all-gathers overlap the remaining sweeps
                for i, (s, l) in enumerate(FCHUNKS):
                    hp = ps_hop.tile([F, l], f32, tag=f"hop{i}", name=f"hp{i}")
                    jn = 0
                    for (j0, ln) in runs:
                        if i == STREAM_I:
                            st = gtp.tile(
                                [P, ln * l], bf16, tag=f"st{ln}", name="st"
                            )
                            nc.sync.dma_start(
                                st[:].rearrange("p (t c) -> p t c", t=ln),
                                gs[i][j0 * P : (j0 + ln) * P, :].rearrange(
                                    "(t p) c -> p t c", p=P
                                ),
                            )
                            g = st
                            base = 0
                        else:
                            if k == 1:
                                nc.sync.dma_start(
                                    pins[i][
                                        :, j0 * l : (j0 + ln) * l
                                    ].rearrange("p (t c) -> p t c", t=ln),
                                    gs[i][j0 * P : (j0 + ln) * P, :].rearrange(
                                        "(t p) c -> p t c", p=P
                                    ),
                                )
                            g = pins[i]
                            base = j0 * l
                        for t in range(ln):
                            nc.tensor.matmul(
                                hp[:],
                                lhsT=v_of(v_cur, j0 + t),
                                rhs=g[:, base + t * l : base + (t + 1) * l],
                                start=(jn == 0),
                                stop=(jn == jcv - 1),
                            )
                            jn += 1
                    # sweep epilogue: copy out, Wp contribution
                    nc.vector.tensor_copy(y_t[:, s : s + l], hp[:])
                    pw = ps_w.tile([F, l], f32, tag="pw")
                    nc.tensor.matmul(
                        pw[:], lhsT=w_sb[:, (k - 1) * F : k * F],
                        rhs=y_t[:, s : s + l], start=True, stop=True,
                    )
                    nc.vector.tensor_add(
                        out_sb[:, s : s + l], out_sb[:, s : s + l], pw[:]
                    )
                    if k < ORDER - 1:
                        # transpose this sweep's rows to natural layout,
                        # cast bf16, partial all-gather; the reload into
                        # the next hop's v happens on SWDGE
                        m0, nm = PARTS[i]
                        stage = sb.tile(
                            [P, nm * F], bf16, tag=f"stage{i}",
                            name=f"stage{i}",
                        )
                        for mm in range(nm):
                            m = m0 + mm
                            tp = ps_tp.tile([P, F], f32, tag="tp", name="tp")
                            nc.tensor.transpose(
                                tp[:], y_t[:, m * P : (m + 1) * P],
                                ident[0:F, 0:F],
                            )
                            nc.vector.tensor_copy(
                                stage[:, mm * F : (mm + 1) * F], tp[:]
                            )
                        cc_in = dram.tile(
                            [P, nm * F], bf16, tag=f"ccin{i}",
                            name=f"ccin{i}",
                        )
                        cc_out = dram.tile(
                            [ncores * P, nm * F], bf16, tag=f"ccout{i}",
                            name=f"ccout{i}",
                        )
                        nc.scalar.dma_start(cc_in[:], stage[:])
                        nc.gpsimd.collective_compute(
                            "AllGather",
                            mybir.AluOpType.bypass,
                            replica_groups=[list(range(ncores))],
                            ins=[cc_in.opt()],
                            outs=[cc_out.opt()],
                        )
                        nc.gpsimd.dma_start(
                            v_next[i][:].rearrange("p (c m) -> p c m", c=ncores),
                            cc_out[:].rearrange("(c p) m -> p c m", p=P),
                        )
                if k < ORDER - 1:
                    v_parts = v_next

            nc.scalar.dma_start(out_t, out_sb[:])

    nc.compile()
    return nc


def get_nc(np_total=NP, ncores=NCORES):
    key = (np_total, ncores)
    if key not in _CACHE:
        _CACHE[key] = _build(np_total, ncores)
    return _CACHE[key]


def prep_inputs(x, gso, weight, np_total=NP, ncores=NCORES):
    """Host-side shard prep. Returns in_maps for run_bass_kernel_spmd."""
    import ml_dtypes

    bf = ml_dtypes.bfloat16
    n = x.shape[0]
    rpc = np_total // ncores
    mc = rpc // P

    x = np.asarray(x, dtype=np.float32)
    gso = np.asarray(gso, dtype=np.float32)
    weight = np.asarray(weight, dtype=np.float32)

    # power-basis weights; k=0 is folded into the host-computed out0
    wp0 = (weight[0] - weight[2]).astype(np.float32)
    wp = np.concatenate(
        [
            weight[1] - 3.0 * weight[3],
            2.0 * weight[2],
            4.0 * weight[3],
        ],
        axis=1,
    ).astype(np.float32)  # [F, (ORDER-1)*F]

    xpad = np.zeros((np_total, F), dtype=np.float32)
    xpad[:n] = x
    out0_full = np.ascontiguousarray((xpad @ wp0).T)  # [F, np_total]

    gpad = np.zeros((np_total, np_total), dtype=np.float32)
    gpad[:n, :n] = gso
    g16 = gpad.astype(bf)

    # x (bf16) in the per-part v layout
    x16 = xpad.astype(bf)

    def part_x(m0, nm):
        return np.ascontiguousarray(
            x16.reshape(ncores, mc, P, F)[:, m0 : m0 + nm]
            .transpose(2, 0, 1, 3)
            .reshape(P, ncores * nm * F)
        )

    xtb = np.ascontiguousarray(
        np.concatenate([part_x(m0, nm) for (m0, nm) in PARTS], axis=1)
    )

    in_maps = []
    for c in range(ncores):
        rows = slice(c * rpc, (c + 1) * rpc)
        gt_c = g16[rows, :].T  # [np_total, rpc] bf16 view
        m = {"xtb": xtb, "wp": wp}
        m["out0"] = np.ascontiguousarray(out0_full[:, rows])
        for i, (s, l) in enumerate(FCHUNKS):
            m[f"g{i}"] = np.ascontiguousarray(gt_c[:, s : s + l])
        in_maps.append(m)
    return in_maps


def assemble_output(results, n=N, ncores=NCORES):
    out_t = np.concatenate([results[c]["outT"] for c in range(ncores)], axis=1)
    return np.ascontiguousarray(out_t.T[:n]).astype(np.float32)


def kernel(x, gso, weight):
    import time

    from concourse import bass_utils

    nc = get_nc()
    in_maps = prep_inputs(x, gso, weight)
    last_err = None
    for attempt in range(3):
        try:
            res = bass_utils.run_bass_kernel_spmd(
                nc, in_maps, core_ids=list(range(NCORES))
            )
            return assemble_output(res.results)
        except Exception as e:  # transient device wedge: retry
            last_err = e
            time.sleep(5.0 * (attempt + 1))
    raise last_err


# revision 4
# speedup vs baseline: 2.1804x; 1.0623x over previous
"""ChebConv (order-4) GNN layer on 8 Trainium2 NeuronCores.

Reference computation (fp32):
    T0 = x, T1 = G x, Tk = 2 G T{k-1} - T{k-2}
    out = sum_k Tk @ W[k]          # [N, F] with N=10000, F=32

Strategy:
  * Rewrite in the power basis: y0 = x, yk = G y{k-1},
      out = sum_k yk @ Wp[k]  with
      Wp = [W0 - W2, W1 - 3 W3, 2 W2, 4 W3]   (exact modulo fp reassociation)
    so each hop is a bare matmul against G (no 2*/- epilogue).
  * Row-shard G over 8 cores (1280 padded rows each; pad N 10000 ->
    10240).  Everything runs in plain bf16 (fp32 PSUM accumulation):
    measured end-to-end relative error ~3.4e-3 against the fp64
    oracle, inside the 2e-2 gate, and one bf16 pass per hop is 3x
    less PE streaming and 2x less HBM than an fp32-accurate scheme.
  * Each hop runs as 3 sweeps over column chunks (512, 384, 384) of
    yk^T, in sweep order (0, 2, 1).  Chunks 0 and 2 stay pinned in
    SBUF across hops (~142KB per partition); chunk 1 streams from
    DRAM each hop, its DMA hidden under the PE time of the two
    pinned sweeps.  Per sweep: matmuls (lhsT=v[j-chunk] [128,32],
    rhs=G^T tile [128,l]) accumulate the sweep's [32,l] chunk of
    yk^T over all 79 valid 128-row j-chunks in one open PSUM
    accumulation group (the all-padding j-chunk 79 is skipped).
  * The host pre-lays G^T out in partition-major supertile format,
    ordered by the kernel's j consumption order: g_i[p, pos*l + c] =
    G^T[jorder[pos]*128 + p, s + c].  Every fill is then a plain 2D
    DMA with 6-8KB contiguous per-partition lines (8 j-chunks per
    descriptor), which keeps the DMA engines near peak instead of
    the ~250GB/s they manage on 0.8KB lines.
  * The k=0 term (x @ Wp0)^T is precomputed on the host and DMAed
    straight into the output accumulator.  The k>=1 Wp contraction
    happens on-chip from yk^T in fp32.
  * After each sweep (except in the last hop), its rows are
    PE-transposed into natural m-chunk layout, cast bf16, and
    all-gathered in a partial collective (DRAM bounce) that overlaps
    the remaining sweeps.  A dummy AllGather issued at kernel start
    absorbs the collectives stream's cold-start barrier (~50us) so
    the first real gather runs warm.  The reload into the next hop's
    per-part v tiles rides SWDGE (gpsimd) so the gather-gated DMA
    cannot convoy the G stream on the shared HWDGE completion lanes;
    j-chunks are consumed in gather-firing order so each hop starts
    on columns whose gather finished first.
  * Output is returned transposed ([32, 1280] per core); the host
    concatenates, transposes and drops padding.
"""

import sys

if "/opt/trn_rl_repo" not in sys.path:
    sys.path.insert(0, "/opt/trn_rl_repo")

import numpy as np

N = 10000
F = 32
ORDER = 4
NCORES = 8
P = 128
NP = 10240  # padded node count: divisible by NCORES * P
RPC = NP // NCORES  # rows per core (1280)
JC = NP // P  # global 128-row chunks (80); the last is all padding
JCV = JC - 1  # valid chunks
MC = RPC // P  # local 128-row chunks per core (10)

# column chunks of the per-core output slice: (start, len)
FCHUNKS = [(0, 512), (512, 384), (896, 384)]
# per-part m-chunk geometry: part i covers m-chunks [m0, m0+nm)
PARTS = [(0, 4), (4, 3), (7, 3)]
STREAM_I = 1  # chunk 1 streams from DRAM each hop; 0 and 2 are pinned
SWEEP_ORDER = [0, 2, 1]  # pinned sweeps first: DMA arrival matches PE use
GRP = 8  # j-chunks per DMA descriptor

_CACHE = {}


def _jorder():
    """j-chunk consumption order: grouped by producing part in sweep
    order (= gather firing order), skipping the all-padding chunk."""
    order = []
    for i in SWEEP_ORDER:
        m0, nm = PARTS[i]
        for c in range(NCORES):
            for mm in range(nm):
                j = c * MC + m0 + mm
                if j != JC - 1:
                    order.append(j)
    return order


def _build(np_total, ncores):
    from concourse import bacc, masks, mybir, tile

    rpc = np_total // ncores
    jc = np_total // P
    mc = rpc // P
    f32 = mybir.dt.float32
    bf16 = mybir.dt.bfloat16
    jcv = jc - 1

    nc = bacc.Bacc(
        "TRN2", target_bir_lowering=False, debug=False, num_devices=ncores
    )
    # per-sweep G^T blocks, host-laid-out partition-major in jorder:
    # g_i[p, pos*l + c] = G^T[jorder[pos]*P + p, s + c]
    gs = [
        nc.dram_tensor(f"g{i}", [P, jcv * l], bf16, kind="ExternalInput").ap()
        for i, (s, l) in enumerate(FCHUNKS)
    ]
    vcols = [ncores * nm * F for (m0, nm) in PARTS]
    # x in per-part v layout: block col (c*nm + ml)*F + f
    #   = padded x row (c*mc + m0 + ml)*P + p
    xtb = nc.dram_tensor("xtb", [P, sum(vcols)], bf16, kind="ExternalInput").ap()
    # host-precomputed k=0 term (x @ Wp0)^T for this core's rows
    out0 = nc.dram_tensor("out0", [F, rpc], f32, kind="ExternalInput").ap()
    wp = nc.dram_tensor("wp", [F, (ORDER - 1) * F], f32, kind="ExternalInput").ap()
    out_t = nc.dram_tensor("outT", [F, rpc], f32, kind="ExternalOutput").ap()

    jorder = _jorder()
    assert len(jorder) == jcv
    groups = [(g0, min(GRP, jcv - g0)) for g0 in range(0, jcv, GRP)]

    def part_of(m):
        for i, (m0, nm) in enumerate(PARTS):
            if m0 <= m < m0 + nm:
                return i
        raise AssertionError

    with tile.TileContext(nc) as tc:
        with (
            tc.tile_pool(name="const", bufs=1) as constp,
            tc.tile_pool(name="gtp", bufs=5) as gtp,
            tc.tile_pool(name="vp", bufs=2) as vp,
            tc.tile_pool(name="sb", bufs=2) as sb,
            tc.tile_pool(name="ps_hop", bufs=1, space="PSUM") as ps_hop,
            tc.tile_pool(name="ps_tp", bufs=2, space="PSUM") as ps_tp,
            tc.tile_pool(name="ps_w", bufs=2, space="PSUM") as ps_w,
            tc.tile_pool(name="dram", bufs=2, space="DRAM") as dram,
        ):
            # dummy gather: absorbs the collectives stream's cold-start
            # barrier concurrently with hop-1 compute
            warm_in = dram.tile([P, F], bf16, tag="warm_in", name="warm_in")
            warm_out = dram.tile(
                [ncores * P, F], bf16, tag="warm_out", name="warm_out"
            )
            nc.gpsimd.collective_compute(
                "AllGather",
                mybir.AluOpType.bypass,
                replica_groups=[list(range(ncores))],
                ins=[warm_in.opt()],
                outs=[warm_out.opt()],
            )

            ident = constp.tile([P, P], f32)
            masks.make_identity(nc, ident[:])
            w_sb = constp.tile([F, (ORDER - 1) * F], f32)
            nc.scalar.dma_start(w_sb[:], wp)
            out_sb = constp.tile([F, rpc], f32)
            nc.scalar.dma_start(out_sb[:], out0)
            pins = {
                i: constp.tile(
                    [P, jcv * FCHUNKS[i][1]], bf16, name=f"pin{i}"
                )
                for i in range(len(FCHUNKS))
                if i != STREAM_I
            }

            # v holds y_{k-1} in bf16, one tile per fc part so next-hop
            # matmuls only depend on the partial gather that produced
            # their columns
            v_parts = []
            off = 0
            for i, w_ in enumerate(vcols):
                vt = vp.tile([P, w_], bf16, tag=f"v{i}", name=f"v{i}")
                nc.scalar.dma_start(vt[:], xtb[:, off : off + w_])
                off += w_
                v_parts.append(vt)

            def v_of(vps, j):
                c, m = j // mc, j % mc
                i = part_of(m)
                m0, nm = PARTS[i]
                col = (c * nm + (m - m0)) * F
                return vps[i][:, col : col + F]

            for k in range(1, ORDER):
                v_cur = v_parts
                if k < ORDER - 1:
                    v_next = [
                        vp.tile([P, w_], bf16, tag=f"v{i}", name=f"vn{i}_{k}")
                        for i, w_ in enumerate(vcols)
                    ]
                y_t = sb.tile([F, rpc], f32, tag="yT")
                # hop: y_k^T = (G @ y_{k-1})^T, one sweep per fc chunk so
                # partial all-gathers overlap the remaining sweeps
                for i in SWEEP_ORDER:
                    s, l = FCHUNKS[i]
                    hp = ps_hop.tile([F, l], f32, tag=f"hop{i}", name=f"hp{i}")
                    for g0, gl in groups:
                        if i == STREAM_I:
                            st = gtp.tile(
                                [P, GRP * l], bf16, tag="st", name="st"
                            )
                            nc.sync.dma_start(
                                st[:, 0 : gl * l],
                                gs[i][:, g0 * l : (g0 + gl) * l],
                            )
                            buf, base = st, 0
                        else:
                            if k == 1:
                                nc.sync.dma_start(
                                    pins[i][:, g0 * l : (g0 + gl) * l],
                                    gs[i][:, g0 * l : (g0 + gl) * l],
                                )
                            buf, base = pins[i], g0 * l
                        for t in range(gl):
                            pos = g0 + t
                            nc.tensor.matmul(
                                hp[:],
                                lhsT=v_of(v_cur, jorder[pos]),
                                rhs=buf[:, base + t * l : base + (t + 1) * l],
                                start=(pos == 0),
                                stop=(pos == jcv - 1),
                            )
                    # sweep epilogue: copy out, Wp contribution
                    nc.vector.tensor_copy(y_t[:, s : s + l], hp[:])
                    pw = ps_w.tile([F, l], f32, tag="pw")
                    nc.tensor.matmul(
                        pw[:], lhsT=w_sb[:, (k - 1) * F : k * F],
                        rhs=y_t[:, s : s + l], start=True, stop=True,
                    )
                    nc.vector.tensor_add(
                        out_sb[:, s : s + l], out_sb[:, s : s + l], pw[:]
                    )
                    if k < ORDER - 1:
                        # transpose this sweep's rows to natural layout,
                        # cast bf16, partial all-gather; the reload into
                        # the next hop's v happens on SWDGE
                        m0, nm = PARTS[i]
                        stage = sb.tile(
                            [P, nm * F], bf16, tag=f"stage{i}",
                            name=f"stage{i}",
                        )
                        for mm in range(nm):
                            m = m0 + mm
                            tp = ps_tp.tile([P, F], f32, tag="tp", name="tp")
                            nc.tensor.transpose(
                                tp[:], y_t[:, m * P : (m + 1) * P],
                                ident[0:F, 0:F],
                            )
                            nc.vector.tensor_copy(
                                stage[:, mm * F : (mm + 1) * F], tp[:]
                            )
                        cc_in = dram.tile(
                            [P, nm * F], bf16, tag=f"ccin{i}",
                            name=f"ccin{i}",
                        )
                        cc_out = dram.tile(
                            [ncores * P, nm * F], bf16, tag=f"ccout{i}",
                            name=f"ccout{i}",
                        )
                        nc.scalar.dma_start(cc_in[:], stage[:])
                        nc.gpsimd.collective_compute(
                            "AllGather",
                            mybir.AluOpType.bypass,
                            replica_groups=[list(range(ncores))],
                            ins=[cc_in.opt()],
                            outs=[cc_out.opt()],
                        )
                        nc.gpsimd.dma_start(
                            v_next[i][:].rearrange("p (c m) -> p c m", c=ncores),
                            cc_out[:].rearrange("(c p) m -> p c m", p=P),
                        )
                if k < ORDER - 1:
                    v_parts = v_next

            nc.scalar.dma_start(out_t, out_sb[:])

    nc.compile()
    return nc


def get_nc(np_total=NP, ncores=NCORES):
    key = (np_total, ncores)
    if key not in _CACHE:
        _CACHE[key] = _build(np_total, ncores)
    return _CACHE[key]


def prep_inputs(x, gso, weight, np_total=NP, ncores=NCORES):
    """Host-side shard prep. Returns in_maps for run_bass_kernel_spmd."""
    import ml_dtypes

    bf = ml_dtypes.bfloat16
    n = x.shape[0]
    rpc = np_total // ncores
    jc = np_total // P
    jcv = jc - 1
    mc = rpc // P

    x = np.asarray(x, dtype=np.float32)
    gso = np.asarray(gso, dtype=np.float32)
    weight = np.asarray(weight, dtype=np.float32)

    # power-basis weights; k=0 is folded into the host-computed out0
    wp0 = (weight[0] - weight[2]).astype(np.float32)
    wp = np.concatenate(
        [
            weight[1] - 3.0 * weight[3],
            2.0 * weight[2],
            4.0 * weight[3],
        ],
        axis=1,
    ).astype(np.float32)  # [F, (ORDER-1)*F]

    xpad = np.zeros((np_total, F), dtype=np.float32)
    xpad[:n] = x
    out0_full = np.ascontiguousarray((xpad @ wp0).T)  # [F, np_total]

    gpad = np.zeros((np_total, np_total), dtype=np.float32)
    gpad[:n, :n] = gso
    g16 = gpad.astype(bf)

    # x (bf16) in the per-part v layout
    x16 = xpad.astype(bf)

    def part_x(m0, nm):
        return np.ascontiguousarray(
            x16.reshape(ncores, mc, P, F)[:, m0 : m0 + nm]
            .transpose(2, 0, 1, 3)
            .reshape(P, ncores * nm * F)
        )

    xtb = np.ascontiguousarray(
        np.concatenate([part_x(m0, nm) for (m0, nm) in PARTS], axis=1)
    )

    jorder = np.asarray(_jorder())
    in_maps = []
    for c in range(ncores):
        rows = slice(c * rpc, (c + 1) * rpc)
        gt_c = g16[rows, :].T  # [np_total, rpc] bf16 view
        m = {"xtb": xtb, "wp": wp}
        m["out0"] = np.ascontiguousarray(out0_full[:, rows])
        for i, (s, l) in enumerate(FCHUNKS):
            # partition-major supertile layout in jorder
            chunk = np.ascontiguousarray(gt_c[:, s : s + l]).reshape(jc, P, l)
            m[f"g{i}"] = np.ascontiguousarray(
                chunk[jorder].transpose(1, 0, 2).reshape(P, jcv * l)
            )
        in_maps.append(m)
    return in_maps


def assemble_output(results, n=N, ncores=NCORES):
    out_t = np.concatenate([results[c]["outT"] for c in range(ncores)], axis=1)
    return np.ascontiguousarray(out_t.T[:n]).astype(np.float32)


def kernel(x, gso, weight):
    import time

    from concourse import bass_utils

    nc = get_nc()
    in_maps = prep_inputs(x, gso, weight)
    last_err = None
    for attempt in range(3):
        try:
            res = bass_utils.run_bass_kernel_spmd(
                nc, in_maps, core_ids=list(range(NCORES))
            )
            return assemble_output(res.results)
        except Exception as e:  # transient device wedge: retry
            last_err = e
            time.sleep(5.0 * (attempt + 1))
    raise last_err


# revision 29
# speedup vs baseline: 2.4426x; 1.1203x over previous
"""ChebConv (order-4) GNN layer on 8 Trainium2 NeuronCores.

Reference computation (fp32):
    T0 = x, T1 = G x, Tk = 2 G T{k-1} - T{k-2}
    out = sum_k Tk @ W[k]          # [N, F] with N=10000, F=32

Strategy:
  * Rewrite in the power basis: y0 = x, yk = G y{k-1},
      out = sum_k yk @ Wp[k]  with
      Wp = [W0 - W2, W1 - 3 W3, 2 W2, 4 W3]   (exact modulo fp reassociation)
    so each hop is a bare matmul against G (no 2*/- epilogue).
  * Row-shard G over 8 cores (1280 padded rows each; pad N 10000 ->
    10240).  Everything runs in plain bf16 (fp32 PSUM accumulation):
    measured end-to-end relative error ~3.4e-3 against the fp64
    oracle, inside the 2e-2 gate, and one bf16 pass per hop is 3x
    less PE streaming and 2x less HBM than an fp32-accurate scheme.
  * The ENTIRE per-core G^T block (~202KB/partition bf16) is pinned
    in SBUF: G is read from HBM exactly once, during hop 1.  Hops 2
    and 3 run pure PE with zero G DMA, which also keeps the shared
    DMA-completion-semaphore rotation free of slow epochs (those
    stalled the collective staging DMAs by 30-50us in streaming
    variants).  To fit: the output accumulates in PSUM (pw matmuls
    with start at k=1, stop at k=3), the k=0 term (x @ Wp0) is added
    by the HOST after gather, and y^T/stage are single-buffered.
  * Each hop runs 3 sweeps over column chunks (512, 384, 384) of
    yk^T.  Per sweep: matmuls (lhsT=v[j-chunk] [128,32], rhs=G^T
    tile [128,l]) accumulate the sweep's [32,l] chunk of yk^T over
    all 79 valid 128-row j-chunks in one open PSUM accumulation
    group (the all-padding j-chunk 79 is skipped).
  * The host pre-lays G^T out partition-major in the kernel's j
    consumption order: g_i[p, pos*l + c] = G^T[jorder[pos]*128 + p,
    s + c].  Hop-1 fills are plain 2D DMAs with 6-8KB contiguous
    per-partition lines (8 j-chunks per descriptor), near peak DMA
    rate, consumed incrementally by the PE.
  * After each sweep (except in the last hop), its rows are
    PE-transposed into natural m-chunk layout, cast bf16, and
    all-gathered in a partial collective (DRAM bounce) that overlaps
    the remaining sweeps.  A dummy AllGather issued at kernel start
    absorbs the collectives stream's cold-start barrier (~50us) so
    the first real gather runs warm.  Collective triggers ride the
    gpsimd queue; the gather-gated v reloads ride the vector queue
    so they cannot convoy the triggers.  j-chunks are consumed in
    gather-firing order so each hop starts on columns whose gather
    finished first.
  * Output is returned transposed ([32, 1280] per core, k>=1 terms
    only); the host concatenates, transposes, drops padding and adds
    the k=0 term.
"""

import sys

if "/opt/trn_rl_repo" not in sys.path:
    sys.path.insert(0, "/opt/trn_rl_repo")

import numpy as np

N = 10000
F = 32
ORDER = 4
NCORES = 8
P = 128
NP = 10240  # padded node count: divisible by NCORES * P
RPC = NP // NCORES  # rows per core (1280)
JC = NP // P  # global 128-row chunks (80); the last is all padding
JCV = JC - 1  # valid chunks
MC = RPC // P  # local 128-row chunks per core (10)

# column chunks of the per-core output slice: (start, len)
FCHUNKS = [(0, 512), (512, 384), (896, 384)]
# per-part m-chunk geometry: part i covers m-chunks [m0, m0+nm)
PARTS = [(0, 4), (4, 3), (7, 3)]
SWEEP_ORDER = [0, 1, 2]
GRP = 8  # j-chunks per hop-1 fill descriptor

_CACHE = {}


def _jorder():
    """j-chunk consumption order: grouped by producing part in sweep
    order (= gather firing order), skipping the all-padding chunk."""
    order = []
    for i in SWEEP_ORDER:
        m0, nm = PARTS[i]
        for c in range(NCORES):
            for mm in range(nm):
                j = c * MC + m0 + mm
                if j != JC - 1:
                    order.append(j)
    return order


def _build(np_total, ncores):
    from concourse import bacc, masks, mybir, tile

    rpc = np_total // ncores
    jc = np_total // P
    mc = rpc // P
    f32 = mybir.dt.float32
    bf16 = mybir.dt.bfloat16
    jcv = jc - 1

    nc = bacc.Bacc(
        "TRN2", target_bir_lowering=False, debug=False, num_devices=ncores
    )
    # per-sweep G^T blocks, host-laid-out partition-major in jorder:
    # g_i[p, pos*l + c] = G^T[jorder[pos]*P + p, s + c]
    gs = [
        nc.dram_tensor(f"g{i}", [P, jcv * l], bf16, kind="ExternalInput").ap()
        for i, (s, l) in enumerate(FCHUNKS)
    ]
    vcols = [ncores * nm * F for (m0, nm) in PARTS]
    # x in per-part v layout: block col (c*nm + ml)*F + f
    #   = padded x row (c*mc + m0 + ml)*P + p
    xtb = nc.dram_tensor("xtb", [P, sum(vcols)], bf16, kind="ExternalInput").ap()
    wp = nc.dram_tensor("wp", [F, (ORDER - 1) * F], bf16, kind="ExternalInput").ap()
    # per-hop partial outputs (y_k @ Wp_k)^T; the host sums them
    outs_t = [
        nc.dram_tensor(f"out{k}T", [F, rpc], bf16, kind="ExternalOutput").ap()
        for k in range(1, ORDER)
    ]

    jorder = _jorder()
    assert len(jorder) == jcv
    groups = [(g0, min(GRP, jcv - g0)) for g0 in range(0, jcv, GRP)]

    def part_of(m):
        for i, (m0, nm) in enumerate(PARTS):
            if m0 <= m < m0 + nm:
                return i
        raise AssertionError

    with tile.TileContext(nc) as tc:
        with (
            tc.tile_pool(name="const", bufs=1) as constp,
            tc.tile_pool(name="vp", bufs=2) as vp,
            tc.tile_pool(name="sb", bufs=1) as sb,
            tc.tile_pool(name="ps_hop", bufs=1, space="PSUM") as ps_hop,
            tc.tile_pool(name="ps_tp", bufs=2, space="PSUM") as ps_tp,
            tc.tile_pool(name="ps_w", bufs=2, space="PSUM") as ps_w,
            tc.tile_pool(name="dram", bufs=2, space="DRAM") as dram,
        ):
            # dummy gather: absorbs the collectives stream's cold-start
            # barrier + first-op setup concurrently with hop-1 compute
            warm_in = dram.tile([P, F], bf16, tag="warm_in", name="warm_in")
            warm_out = dram.tile(
                [ncores * P, F], bf16, tag="warm_out", name="warm_out"
            )
            nc.gpsimd.collective_compute(
                "AllGather",
                mybir.AluOpType.bypass,
                replica_groups=[list(range(ncores))],
                ins=[warm_in.opt()],
                outs=[warm_out.opt()],
            )

            ident = constp.tile([F, F], bf16)
            masks.make_identity(nc, ident[:])
            w_sb = constp.tile([F, (ORDER - 1) * F], bf16)
            nc.scalar.dma_start(w_sb[:], wp)
            pins = [
                constp.tile([P, jcv * l], bf16, name=f"pin{i}")
                for i, (s, l) in enumerate(FCHUNKS)
            ]

            # v holds y_{k-1} in bf16, one tile per fc part so next-hop
            # matmuls only depend on the partial gather that produced
            # their columns
            v_parts = []
            off = 0
            for i, w_ in enumerate(vcols):
                if i < 2:
                    vt = vp.tile([P, w_], bf16, tag=f"v{i}", name=f"v{i}")
                else:
                    vt = constp.tile([P, w_], bf16, name=f"v{i}")
                nc.scalar.dma_start(vt[:], xtb[:, off : off + w_])
                off += w_
                v_parts.append(vt)

            def v_of(vps, j):
                c, m = j // mc, j % mc
                i = part_of(m)
                m0, nm = PARTS[i]
                col = (c * nm + (m - m0)) * F
                return vps[i][:, col : col + F]

            for k in range(1, ORDER):
                v_cur = v_parts
                if k < ORDER - 1:
                    # parts 0/1 double-buffer so their reloads can land
                    # mid-hop; part 2's gather only completes at hop end
                    # anyway, so its reload overwrites the tile in place
                    v_next = [
                        vp.tile([P, w_], bf16, tag=f"v{i}", name=f"vn{i}_{k}")
                        if i < 2
                        else v_parts[2]
                        for i, w_ in enumerate(vcols)
                    ]
                # hop: y_k^T = (G @ y_{k-1})^T, one sweep per fc chunk so
                # partial all-gathers overlap the remaining sweeps
                for i in SWEEP_ORDER:
                    s, l = FCHUNKS[i]
                    y_t = sb.tile([F, l], bf16, tag="yT", name="yT")
                    hp = ps_hop.tile([F, l], f32, tag=f"hop{i}", name=f"hp{i}")
                    for g0, gl in groups:
                        if k == 1:
                            nc.sync.dma_start(
                                pins[i][:, g0 * l : (g0 + gl) * l],
                                gs[i][:, g0 * l : (g0 + gl) * l],
                            )
                        for t in range(gl):
                            pos = g0 + t
                            nc.tensor.matmul(
                                hp[:],
                                lhsT=v_of(v_cur, jorder[pos]),
                                rhs=pins[i][:, pos * l : (pos + 1) * l],
                                start=(pos == 0),
                                stop=(pos == jcv - 1),
                            )
                    # sweep epilogue: copy out, Wp contribution
                    nc.vector.tensor_copy(y_t[:], hp[:])
                    pw = ps_w.tile([F, l], f32, tag="pw", name="pw")
                    nc.tensor.matmul(
                        pw[:], lhsT=w_sb[:, (k - 1) * F : k * F],
                        rhs=y_t[:], start=True, stop=True,
                    )
                    if k < ORDER - 1:
                        # transpose this sweep's rows to natural layout,
                        # cast bf16, partial all-gather; v reloads ride
                        # the vector queue so they can't convoy the
                        # gpsimd collective triggers
                        m0, nm = PARTS[i]
                        stage = sb.tile(
                            [P, 4 * F], bf16, tag="stage", name="stage"
                        )
                        for mm in range(nm):
                            tp = ps_tp.tile([P, F], bf16, tag="tp", name="tp")
                            nc.tensor.transpose(
                                tp[:], y_t[:, mm * P : (mm + 1) * P],
                                ident[:],
                            )
                            nc.vector.tensor_copy(
                                stage[:, mm * F : (mm + 1) * F], tp[:]
                            )
                        cc_in = dram.tile(
                            [P, nm * F], bf16, tag=f"ccin{i}",
                            name=f"ccin{i}",
                        )
                        cc_out = dram.tile(
                            [ncores * P, nm * F], bf16, tag=f"ccout{i}",
                            name=f"ccout{i}",
                        )
                        nc.scalar.dma_start(cc_in[:], stage[:, 0 : nm * F])
                        nc.gpsimd.collective_compute(
                            "AllGather",
                            mybir.AluOpType.bypass,
                            replica_groups=[list(range(ncores))],
                            ins=[cc_in.opt()],
                            outs=[cc_out.opt()],
                        )
                        nc.sync.dma_start(
                            v_next[i][:].rearrange("p (c m) -> p c m", c=ncores),
                            cc_out[:].rearrange("(c p) m -> p c m", p=P),
                        )
                    # fold this hop's output partial back through y_t
                    # (free once the transposes are done), bf16, ship it
                    nc.vector.tensor_copy(y_t[:], pw[:])
                    nc.scalar.dma_start(outs_t[k - 1][:, s : s + l], y_t[:])
                if k < ORDER - 1:
                    v_parts = v_next

    nc.compile()
    return nc


def get_nc(np_total=NP, ncores=NCORES):
    key = (np_total, ncores)
    if key not in _CACHE:
        _CACHE[key] = _build(np_total, ncores)
    return _CACHE[key]


def prep_inputs(x, gso, weight, np_total=NP, ncores=NCORES):
    """Host-side shard prep. Returns in_maps for run_bass_kernel_spmd."""
    import ml_dtypes

    bf = ml_dtypes.bfloat16
    n = x.shape[0]
    rpc = np_total // ncores
    jc = np_total // P
    jcv = jc - 1
    mc = rpc // P

    x = np.asarray(x, dtype=np.float32)
    gso = np.asarray(gso, dtype=np.float32)
    weight = np.asarray(weight, dtype=np.float32)

    # power-basis weights for k>=1; k=0 is added on the host
    wp = np.concatenate(
        [
            weight[1] - 3.0 * weight[3],
            2.0 * weight[2],
            4.0 * weight[3],
        ],
        axis=1,
    ).astype(bf)  # [F, (ORDER-1)*F]

    xpad = np.zeros((np_total, F), dtype=np.float32)
    xpad[:n] = x

    gpad = np.zeros((np_total, np_total), dtype=np.float32)
    gpad[:n, :n] = gso
    g16 = gpad.astype(bf)

    # x (bf16) in the per-part v layout
    x16 = xpad.astype(bf)

    def part_x(m0, nm):
        return np.ascontiguousarray(
            x16.reshape(ncores, mc, P, F)[:, m0 : m0 + nm]
            .transpose(2, 0, 1, 3)
            .reshape(P, ncores * nm * F)
        )

    xtb = np.ascontiguousarray(
        np.concatenate([part_x(m0, nm) for (m0, nm) in PARTS], axis=1)
    )

    jorder = np.asarray(_jorder())
    in_maps = []
    for c in range(ncores):
        rows = slice(c * rpc, (c + 1) * rpc)
        gt_c = g16[rows, :].T  # [np_total, rpc] bf16 view
        m = {"xtb": xtb, "wp": wp}
        for i, (s, l) in enumerate(FCHUNKS):
            # partition-major supertile layout in jorder
            chunk = np.ascontiguousarray(gt_c[:, s : s + l]).reshape(jc, P, l)
            m[f"g{i}"] = np.ascontiguousarray(
                chunk[jorder].transpose(1, 0, 2).reshape(P, jcv * l)
            )
        in_maps.append(m)
    return in_maps


def assemble_output(results, x, weight, n=N, ncores=NCORES):
    out_t = sum(
        np.concatenate(
            [results[c][f"out{k}T"] for c in range(ncores)], axis=1
        ).astype(np.float32)
        for k in range(1, ORDER)
    )
    out = np.ascontiguousarray(out_t.T[:n])
    # k=0 term, host-side in fp32
    wp0 = (weight[0] - weight[2]).astype(np.float32)
    out += np.asarray(x, dtype=np.float32) @ wp0
    return out


def kernel(x, gso, weight):
    import time

    from concourse import bass_utils

    nc = get_nc()
    in_maps = prep_inputs(x, gso, weight)
    last_err = None
    for attempt in range(3):
        try:
            res = bass_utils.run_bass_kernel_spmd(
                nc, in_maps, core_ids=list(range(NCORES))
            )
            return assemble_output(res.results, x, weight)
        except Exception as e:  # transient device wedge: retry
            last_err = e
            time.sleep(5.0 * (attempt + 1))
    raise last_err
